# revision 29
# baseline (speedup 1.0000x reference)
"""Trainium2 Bass kernel for the ESN (echo state network) forward scan.

  x_{t+1} = (1-a) x_t + a tanh(u_t + x_t @ W),  a = 0.5
  U = einsum('bit,in->tbn', Input, W_in);  out X[b,n,t] = x_{t+1}[b,n]

Sharding: data-parallel over batch (B=64 -> 8 cores x 8 batches).
W, W_in replicated; no collectives. Each core runs the full T=2000 scan
for its 8 batches and writes its [8, 1024, 2000] output slice.

v3 structure (per core, per step):
 - z matmuls: 4 concurrent PE column groups, one per 128-col n-slice of
   each 512-col half; group j accumulates u + 8 k-tile matmuls (F=128)
   at PSUM rows 32j..32j+8 -> z packs into a [128, 128] PSUM bank/half.
 - ACT: h16 = tanh(z) applied in the UNTRANSPOSED [b, n] layout
   (elementwise tanh commutes with the selection-matrix transpose), so
   there is no separate PSUM->SBUF copy; 2 ACT ops per step total.
 - PE: hT[p_n, (g,b)] = h16.T @ sel, sel[32g+b, 8g+b] = 1 -- one matmul
   per half transposes tanh back to the state layout.
 - DVE (all fp16 state, no fp32 master):
     xh   = 0.5 * s16        (early, off critical path)
     s16' = xh + hT          (the only chain op; next matmul operand)
     obuf[t] = 0.5 * s16'    (= x_{t+1}, written strided, f32)
 - Filler matmuls into a scratch PSUM bank plug the per-step PE idle
   gaps (plus a startup burst) so the PE HAM clock-gate stays at 2.4GHz
   (K=8/8); with idle gaps the whole kernel measures at 1.2GHz.
Output chunks of TC steps buffered in SBUF, DMA'd per 128-row g-block.
"""

import os
import numpy as np

import concourse.bass as bass
import concourse.mybir as mybir
import concourse.tile as tile
from concourse.bass import ds
from concourse.bass_utils import run_bass_kernel_spmd

FP32 = mybir.dt.float32
FP16 = mybir.dt.float16

ALPHA = 0.5
N_CORES = 8
B, N_IN, T, N = 64, 16, 2000, 1024
TC = 100  # steps buffered per output chunk
FILLERS_PER_STEP = 4
WARMUP_MMS = 40

LAST_EXEC_NS = None
_CACHED_NC = None


def _split_excess_waits(nc, limit=1):
    """The walrus build in this container rejects instructions carrying more
    than one sem wait; hoist extra waits onto same-engine NoOps."""
    import bass_rust
    for f in nc.m.functions:
        for bb in f.blocks:
            new_insts = []
            for ins in bb.instructions:
                si = ins.sync_info
                if si is not None and si.on_wait and len(si.on_wait) > limit:
                    waits = list(si.on_wait)
                    head, tail = waits[:-limit], waits[-limit:]
                    for j, w in enumerate(head):
                        c = bass_rust.InstNoOp(name=f"{ins.name}-w{j}")
                        c.engine = ins.engine
                        c.sync_info = mybir.SyncInfo(on_wait=[w], on_update=[])
                        new_insts.append(c)
                    si.on_wait = tail
                new_insts.append(ins)
            bb.instructions = new_insts
    return nc


def _batch_pe_incs(nc, stride=4):
    """Serialized EVT_SEM writes (~26ns each) cap the PE matmul issue rate
    at ~34ns/MM. Batch the per-matmul +1 completion increments: every
    `stride`-th matmul carries update_value=stride instead. Cumulative sem
    counts match the original exactly at group boundaries and are never
    ahead of it in between, so every waiter fires at most stride-1 matmul
    completions late and no wait threshold or loop reset needs rewriting."""
    for f in nc.m.functions:
        for bb in f.blocks:
            pend = []  # deferred (update-object, ) count
            last_mm_si = None
            for ins in bb.instructions:
                if type(ins).__name__ != 'InstMatmult':
                    continue
                si = ins.sync_info
                if si is None or not si.on_update:
                    continue
                ups = list(si.on_update)
                if len(ups) != 1:
                    continue
                u = ups[0]
                if getattr(u, 'update_mode', None) != 'sem-inc' or \
                        getattr(u, 'update_value', None) != 1:
                    continue
                pend.append(u)
                last_mm_si = si
                if len(pend) >= stride:
                    # this MM keeps its update, carrying the whole group
                    u.update_value = len(pend)
                    pend = []
                else:
                    si.on_update = []  # strip; a later MM carries the count
            # flush: if the block ended mid-group, restore the last MM's
            # update carrying the residual count
            if pend and last_mm_si is not None:
                last_mm_si.on_update = [pend[-1]]
                pend[-1].update_value = len(pend)
    return nc


def _build_nc(n=N, t_total=T, tc_steps=TC, n_in=N_IN, bc=B // N_CORES):
    G = n // 128          # 8 global 128-row n-blocks
    GH = G // 2           # 4 col groups per 512-col half
    KT = n // 128         # 8 k-tiles of the contraction
    n_chunks = t_total // tc_steps

    assert G == 8 and bc == 8

    nc = bass.Bass()
    sel_dram = nc.dram_tensor("sel", [128, 32], FP16, kind="ExternalInput")
    # w[p, (k, h, j, c)] = 0.5 * W[128k + p, 512h + 128j + c]
    w_dram = nc.dram_tensor("w", [128, KT * n], FP16, kind="ExternalInput")
    win_dram = nc.dram_tensor("win", [n_in, n], FP16, kind="ExternalInput")
    inpT_dram = nc.dram_tensor("inpT", [n_in, t_total, bc], FP16,
                               kind="ExternalInput")
    x_dram = nc.dram_tensor("xout", [bc, n, t_total], FP32,
                            kind="ExternalOutput")
    x_dram_r = x_dram.rearrange("b (g p) t -> p g b t", p=128)

    def w_off(k, h, j):
        return ((k * 2 + h) * GH + j) * 128

    with tile.TileContext(nc) as tc:
        with (
            tc.tile_pool(name="const", bufs=1) as const_pool,
            tc.tile_pool(name="state", bufs=1) as state_pool,
            tc.tile_pool(name="work", bufs=3) as work_pool,
            tc.tile_pool(name="obuf", bufs=2) as obuf_pool,
            tc.tile_pool(name="inp", bufs=2) as inp_pool,
            tc.tile_pool(name="psum", bufs=1, space="PSUM") as psum_pool,
            tc.tile_pool(name="psumS", bufs=1, space="PSUM") as psum_static,
        ):
            w_sb = const_pool.tile([128, KT * n], FP16)
            nc.sync.dma_start(w_sb[:, :], w_dram[:, :])
            win_sb = const_pool.tile([n_in, n], FP16)
            nc.sync.dma_start(win_sb[:, :], win_dram[:, :])
            sel_sb = const_pool.tile([128, 32], FP16)
            nc.sync.dma_start(sel_sb[:, :], sel_dram[:, :])
            zero16 = const_pool.tile([128, 128], FP16)
            nc.vector.memset(zero16[:, :], 0.0)

            # 4 static psum banks (2 halves x ping-pong); zero-filled once so
            # never-written partition rows stay finite zeros (tanh reads all
            # 128 rows; tanh(0)=0 keeps the sel contraction exact)
            zpsS = [[psum_static.tile([128, 128], FP32, name=f"zps_{h_}_{b_}")
                     for b_ in range(2)] for h_ in range(2)]
            for h_ in range(2):
                for b_ in range(2):
                    nc.tensor.matmul(
                        zpsS[h_][b_][:, :], zero16[:, :], zero16[:, :],
                        start=True, stop=True, skip_group_check=True)
            # scratch bank for HAM-warming filler matmuls
            scr = psum_static.tile([128, 128], FP32, name="scratch")

            def filler(tag, cnt, fdim=8):
                # fdim=8 steady-state fillers keep the PE busy for HAM while
                # streaming 16x fewer SBUF columns than F=128 (the z loop is
                # SBUF-stream-bound, so filler traffic is not free)
                for i_ in range(cnt):
                    nc.tensor.matmul(
                        scr[0:8, 0:fdim], zero16[:, 0:8], zero16[:, 0:fdim],
                        start=True, stop=True, skip_group_check=True)

            filler("warm", WARMUP_MMS, fdim=128)

            # State: fp16 only, layout [128 p_n, (g in 0..3, b in 0..7)]
            # with n = 512h + 128g + p; s16 = x_t + h_t (the matmul operand;
            # the 0.5 leak is folded into W host-side). xh = 0.5*s16 = x_t.
            s16s = [[state_pool.tile([128, GH * 8], FP16, name=f"s16_{b_}_{h_}")
                     for h_ in range(2)] for b_ in range(2)]
            xhs = [state_pool.tile([128, GH * 8], FP16, name=f"xh_{h_}")
                   for h_ in range(2)]
            for b_ in range(2):
                for h_ in range(2):
                    nc.vector.memset(s16s[b_][h_][:, :], 0.0)
            for h_ in range(2):
                nc.vector.memset(xhs[h_][:, :], 0.0)

            def chunk_body(ci):
                inp_sb = inp_pool.tile([n_in, tc_steps * bc], FP16)
                nc.sync.dma_start(
                    inp_sb[:, :], inpT_dram[:, ds(ci * tc_steps, tc_steps), :])
                obuf = obuf_pool.tile([128, G * 8 * tc_steps], FP32)
                obuf_r = obuf[:, :].rearrange(
                    "p (g b t) -> p g b t", g=G, b=8, t=tc_steps)

                def emit_u(t):
                    # u for step t opens (start=True) rows 32j..32j+8 of the
                    # static psum banks; the z k-tile partials land on top.
                    # Measured on HW: per-group start=True does NOT wipe the
                    # other groups' writes (clear is per written region).
                    zp = [zpsS[h][t % 2] for h in range(2)]
                    for h in range(2):
                        for j in range(GH):
                            nc.tensor.matmul(
                                zp[h][32 * j: 32 * j + 8, :],
                                inp_sb[:, t * bc: (t + 1) * bc],
                                win_sb[:, 512 * h + 128 * j:
                                       512 * h + 128 * j + 128],
                                start=True, stop=False,
                                skip_group_check=True,
                                tile_position=(0, 32 * j),
                            )
                    return zp

                def emit_z(zps, s16, h):
                    # col group j computes z[:, 512h+128j ..+128]; the 4
                    # groups stream concurrently (distinct col_grp masks)
                    for k in range(KT):
                        for j in range(GH):
                            nc.tensor.matmul(
                                zps[h][32 * j: 32 * j + 8, :],
                                s16[k // GH][:, (k % GH) * 8: (k % GH) * 8 + 8],
                                w_sb[:, w_off(k, h, j): w_off(k, h, j) + 128],
                                start=False, stop=(k == KT - 1),
                                skip_group_check=True,
                                tile_position=(0, 32 * j),
                            )

                def emit_tanh(zps, h):
                    # ACT: tanh in the untransposed layout, fp16 out
                    h16 = work_pool.tile([128, 128], FP16, tag=f"h16_{h}",
                                         name=f"h16_{h}")
                    nc.scalar.activation(
                        h16[:, :], zps[h][:, :],
                        mybir.ActivationFunctionType.Tanh)
                    return h16

                def emit_sel(h16, h):
                    # PE: transpose back to state layout via selection matmul
                    hTp = psum_pool.tile([128, GH * 8], FP32,
                                         tag=f"hT{h}", name=f"hTp{h}")
                    nc.tensor.matmul(
                        hTp[:, :], h16[:, :], sel_sb[:, :],
                        start=True, stop=True, skip_group_check=True)
                    return hTp

                def emit_update(hTp, t, h):
                    s16_n = s16s[(t + 1) % 2][h]
                    nc.vector.tensor_add(s16_n[:, :], xhs[h][:, :], hTp[:, :])
                    s_r = s16_n[:, :].rearrange("p (g b) -> p g b", g=GH, b=8)
                    nc.vector.tensor_scalar_mul(
                        obuf_r[:, GH * h: GH * (h + 1), :, t],
                        s_r[:, :, :], ALPHA)

                def emit_xh(t, h):
                    # xh = 0.5 * s16 = x_t; consumed by s16' add after tanh
                    nc.vector.tensor_scalar_mul(
                        xhs[h][:, :], s16s[t % 2][h][:, :], ALPHA)

                zps_cur = emit_u(0)
                for t in range(tc_steps):
                    s16 = s16s[t % 2]
                    zps = zps_cur
                    emit_xh(t, 0)
                    emit_xh(t, 1)
                    emit_z(zps, s16, 0)
                    h16_0 = emit_tanh(zps, 0)     # ACT, runs under z h1
                    emit_z(zps, s16, 1)
                    h16_1 = emit_tanh(zps, 1)
                    # PE order: z h0 | z h1 | sel h0 | u | fillers | sel h1
                    hTp0 = emit_sel(h16_0, 0)
                    emit_update(hTp0, t, 0)
                    if t + 1 < tc_steps:
                        zps_cur = emit_u(t + 1)
                    filler(f"s{t}", FILLERS_PER_STEP)
                    hTp1 = emit_sel(h16_1, 1)
                    emit_update(hTp1, t, 1)

                for g in range(G):
                    nc.sync.dma_start(
                        x_dram_r[:, g, :, ds(ci * tc_steps, tc_steps)],
                        obuf_r[:, g, :, :],
                    )

            with tc.For_i(0, n_chunks, 1) as i:
                chunk_body(i)

    _split_excess_waits(nc)
    return nc


def kernel(Input, W_in, W):
    """Full inputs in, full output out. Shards batch over 8 NeuronCores."""
    global LAST_EXEC_NS, _CACHED_NC
    Input = np.ascontiguousarray(np.asarray(Input, dtype=np.float32))
    W_in = np.ascontiguousarray(np.asarray(W_in, dtype=np.float32))
    W = np.ascontiguousarray(np.asarray(W, dtype=np.float32))
    Bf, n_in, t_total = Input.shape
    n = W.shape[0]
    bc = Bf // N_CORES

    tc_steps = TC if t_total % TC == 0 else max(
        d for d in range(1, min(TC, t_total) + 1) if t_total % d == 0)
    if _CACHED_NC is None:
        _CACHED_NC = _build_nc(n=n, t_total=t_total, tc_steps=tc_steps,
                               n_in=n_in, bc=bc)
    nc = _CACHED_NC

    # leak factor folded into W: matmul operand is s = x + h = 2x, so W/2.
    # layout [p, (k, h, j, c)] = 0.5*W[128k+p, 512h+128j+c]
    w_r = np.ascontiguousarray(
        (ALPHA * W).reshape(8, 128, 2, 4, 128).transpose(1, 0, 2, 3, 4)
        .reshape(128, 8 * n)
    ).astype(np.float16)
    win16 = W_in.astype(np.float16)
    sel = np.zeros((128, 32), dtype=np.float16)
    for g_ in range(4):
        for b_ in range(8):
            sel[32 * g_ + b_, 8 * g_ + b_] = 1.0
    in_maps = []
    for c in range(N_CORES):
        inpT = np.ascontiguousarray(
            Input[c * bc:(c + 1) * bc].transpose(1, 2, 0)).astype(np.float16)
        in_maps.append({"w": w_r, "win": win16, "inpT": inpT, "sel": sel})

    trace = bool(int(os.environ.get("ESN_TRACE", "0")))
    res = run_bass_kernel_spmd(
        nc, in_maps, core_ids=list(range(N_CORES)), trace=trace)
    LAST_EXEC_NS = res.exec_time_ns

    out = np.concatenate([res.results[c]["xout"] for c in range(N_CORES)],
                         axis=0)
    return np.ascontiguousarray(out.astype(np.float32))


# revision 31
# speedup vs baseline: 1.0560x; 1.0560x over previous
"""Trainium2 Bass kernel for the ESN (echo state network) forward scan.

  x_{t+1} = (1-a) x_t + a tanh(u_t + x_t @ W),  a = 0.5
  U = einsum('bit,in->tbn', Input, W_in);  out X[b,n,t] = x_{t+1}[b,n]

Sharding: data-parallel over batch (B=64 -> 8 cores x 8 batches).
W, W_in replicated; no collectives. Each core runs the full T=2000 scan
for its 8 batches and writes its [8, 1024, 2000] output slice.

v3 structure (per core, per step):
 - z matmuls: 4 concurrent PE column groups, one per 128-col n-slice of
   each 512-col half; group j accumulates u + 8 k-tile matmuls (F=128)
   at PSUM rows 32j..32j+8 -> z packs into a [128, 128] PSUM bank/half.
 - ACT: h16 = tanh(z) applied in the UNTRANSPOSED [b, n] layout
   (elementwise tanh commutes with the selection-matrix transpose), so
   there is no separate PSUM->SBUF copy; 2 ACT ops per step total.
 - PE: hT[p_n, (g,b)] = h16.T @ sel, sel[32g+b, 8g+b] = 1 -- one matmul
   per half transposes tanh back to the state layout.
 - DVE (all fp16 state, no fp32 master):
     xh   = 0.5 * s16        (early, off critical path)
     s16' = xh + hT          (the only chain op; next matmul operand)
     obuf[t] = 0.5 * s16'    (= x_{t+1}, written strided, f32)
 - Filler matmuls into a scratch PSUM bank plug the per-step PE idle
   gaps (plus a startup burst) so the PE HAM clock-gate stays at 2.4GHz
   (K=8/8); with idle gaps the whole kernel measures at 1.2GHz.
Output chunks of TC steps buffered in SBUF, DMA'd per 128-row g-block.
"""

import os
import numpy as np

import concourse.bass as bass
import concourse.mybir as mybir
import concourse.tile as tile
from concourse.bass import ds
from concourse.bass_utils import run_bass_kernel_spmd

FP32 = mybir.dt.float32
FP16 = mybir.dt.float16

ALPHA = 0.5
N_CORES = 8
B, N_IN, T, N = 64, 16, 2000, 1024
TC = 100  # steps buffered per output chunk
FILLERS_PER_STEP = 4
WARMUP_MMS = 40

LAST_EXEC_NS = None
_CACHED_NC = None


def _split_excess_waits(nc, limit=1):
    """The walrus build in this container rejects instructions carrying more
    than one sem wait; hoist extra waits onto same-engine NoOps."""
    import bass_rust
    for f in nc.m.functions:
        for bb in f.blocks:
            new_insts = []
            for ins in bb.instructions:
                si = ins.sync_info
                if si is not None and si.on_wait and len(si.on_wait) > limit:
                    waits = list(si.on_wait)
                    head, tail = waits[:-limit], waits[-limit:]
                    for j, w in enumerate(head):
                        c = bass_rust.InstNoOp(name=f"{ins.name}-w{j}")
                        c.engine = ins.engine
                        c.sync_info = mybir.SyncInfo(on_wait=[w], on_update=[])
                        new_insts.append(c)
                    si.on_wait = tail
                new_insts.append(ins)
            bb.instructions = new_insts
    return nc


def _batch_pe_incs(nc, stride=4):
    """Serialized EVT_SEM writes (~26ns each) cap the PE matmul issue rate
    at ~34ns/MM. Batch the per-matmul +1 completion increments: every
    `stride`-th matmul carries update_value=stride instead. Cumulative sem
    counts match the original exactly at group boundaries and are never
    ahead of it in between, so every waiter fires at most stride-1 matmul
    completions late and no wait threshold or loop reset needs rewriting."""
    for f in nc.m.functions:
        for bb in f.blocks:
            pend = []  # deferred (update-object, ) count
            last_mm_si = None
            for ins in bb.instructions:
                if type(ins).__name__ != 'InstMatmult':
                    continue
                si = ins.sync_info
                if si is None or not si.on_update:
                    continue
                ups = list(si.on_update)
                if len(ups) != 1:
                    continue
                u = ups[0]
                if getattr(u, 'update_mode', None) != 'sem-inc' or \
                        getattr(u, 'update_value', None) != 1:
                    continue
                pend.append(u)
                last_mm_si = si
                if len(pend) >= stride:
                    # this MM keeps its update, carrying the whole group
                    u.update_value = len(pend)
                    pend = []
                else:
                    si.on_update = []  # strip; a later MM carries the count
            # flush: if the block ended mid-group, restore the last MM's
            # update carrying the residual count
            if pend and last_mm_si is not None:
                last_mm_si.on_update = [pend[-1]]
                pend[-1].update_value = len(pend)
    return nc


def _build_nc(n=N, t_total=T, tc_steps=TC, n_in=N_IN, bc=B // N_CORES):
    G = n // 128          # 8 global 128-row n-blocks
    GH = G // 2           # 4 col groups per 512-col half
    KT = n // 128         # 8 k-tiles of the contraction
    n_chunks = t_total // tc_steps

    assert G == 8 and bc == 8

    nc = bass.Bass()
    sel_dram = nc.dram_tensor("sel", [128, 32], FP16, kind="ExternalInput")
    # w[p, (k, h, j, c)] = 0.5 * W[128k + p, 512h + 128j + c]
    w_dram = nc.dram_tensor("w", [128, KT * n], FP16, kind="ExternalInput")
    win_dram = nc.dram_tensor("win", [n_in, n], FP16, kind="ExternalInput")
    inpT_dram = nc.dram_tensor("inpT", [n_in, t_total, bc], FP16,
                               kind="ExternalInput")
    x_dram = nc.dram_tensor("xout", [bc, n, t_total], FP32,
                            kind="ExternalOutput")
    x_dram_r = x_dram.rearrange("b (g p) t -> p g b t", p=128)

    def w_off(k, h, j):
        return ((k * 2 + h) * GH + j) * 128

    with tile.TileContext(nc) as tc:
        with (
            tc.tile_pool(name="const", bufs=1) as const_pool,
            tc.tile_pool(name="state", bufs=1) as state_pool,
            tc.tile_pool(name="work", bufs=3) as work_pool,
            tc.tile_pool(name="obuf", bufs=2) as obuf_pool,
            tc.tile_pool(name="inp", bufs=2) as inp_pool,
            tc.tile_pool(name="psum", bufs=1, space="PSUM") as psum_pool,
            tc.tile_pool(name="psumS", bufs=1, space="PSUM") as psum_static,
        ):
            w_sb = const_pool.tile([128, KT * n], FP16)
            nc.sync.dma_start(w_sb[:, :], w_dram[:, :])
            win_sb = const_pool.tile([n_in, n], FP16)
            nc.sync.dma_start(win_sb[:, :], win_dram[:, :])
            sel_sb = const_pool.tile([128, 32], FP16)
            nc.sync.dma_start(sel_sb[:, :], sel_dram[:, :])
            zero16 = const_pool.tile([128, 128], FP16)
            nc.vector.memset(zero16[:, :], 0.0)

            # 4 static psum banks (2 halves x ping-pong); zero-filled once so
            # never-written partition rows stay finite zeros (tanh reads all
            # 128 rows; tanh(0)=0 keeps the sel contraction exact)
            zpsS = [[psum_static.tile([128, 128], FP32, name=f"zps_{h_}_{b_}")
                     for b_ in range(2)] for h_ in range(2)]
            for h_ in range(2):
                for b_ in range(2):
                    nc.tensor.matmul(
                        zpsS[h_][b_][:, :], zero16[:, :], zero16[:, :],
                        start=True, stop=True, skip_group_check=True)
            # scratch bank for HAM-warming filler matmuls
            scr = psum_static.tile([128, 128], FP32, name="scratch")

            def filler(tag, cnt):
                for i_ in range(cnt):
                    nc.tensor.matmul(
                        scr[0:8, :], zero16[:, 0:8], zero16[:, :],
                        start=True, stop=True, skip_group_check=True)

            filler("warm", WARMUP_MMS)

            # State: fp16 only, layout [128 p_n, (g in 0..3, b in 0..7)]
            # with n = 512h + 128g + p; s16 = x_t + h_t (the matmul operand;
            # the 0.5 leak is folded into W host-side). xh = 0.5*s16 = x_t.
            s16s = [[state_pool.tile([128, GH * 8], FP16, name=f"s16_{b_}_{h_}")
                     for h_ in range(2)] for b_ in range(2)]
            xhs = [state_pool.tile([128, GH * 8], FP16, name=f"xh_{h_}")
                   for h_ in range(2)]
            for b_ in range(2):
                for h_ in range(2):
                    nc.vector.memset(s16s[b_][h_][:, :], 0.0)
            for h_ in range(2):
                nc.vector.memset(xhs[h_][:, :], 0.0)

            def chunk_body(ci):
                inp_sb = inp_pool.tile([n_in, tc_steps * bc], FP16)
                nc.sync.dma_start(
                    inp_sb[:, :], inpT_dram[:, ds(ci * tc_steps, tc_steps), :])
                obuf = obuf_pool.tile([128, G * 8 * tc_steps], FP32)
                obuf_r = obuf[:, :].rearrange(
                    "p (g b t) -> p g b t", g=G, b=8, t=tc_steps)

                def emit_u(t):
                    # u for step t opens (start=True) rows 32j..32j+8 of the
                    # static psum banks; the z k-tile partials land on top.
                    # Measured on HW: per-group start=True does NOT wipe the
                    # other groups' writes (clear is per written region).
                    zp = [zpsS[h][t % 2] for h in range(2)]
                    for h in range(2):
                        for j in range(GH):
                            nc.tensor.matmul(
                                zp[h][32 * j: 32 * j + 8, :],
                                inp_sb[:, t * bc: (t + 1) * bc],
                                win_sb[:, 512 * h + 128 * j:
                                       512 * h + 128 * j + 128],
                                start=True, stop=False,
                                skip_group_check=True,
                                tile_position=(0, 32 * j),
                            )
                    return zp

                def emit_z(zps, s16, h):
                    # col group j computes z[:, 512h+128j ..+128]; the 4
                    # groups stream concurrently (distinct col_grp masks)
                    for k in range(KT):
                        for j in range(GH):
                            nc.tensor.matmul(
                                zps[h][32 * j: 32 * j + 8, :],
                                s16[k // GH][:, (k % GH) * 8: (k % GH) * 8 + 8],
                                w_sb[:, w_off(k, h, j): w_off(k, h, j) + 128],
                                start=False, stop=(k == KT - 1),
                                skip_group_check=True,
                                tile_position=(0, 32 * j),
                            )

                def emit_tanh(zps, h):
                    # ACT: tanh in the untransposed layout, fp16 out.
                    # high_priority: the Tile list scheduler otherwise pops
                    # this after every lower-priority (earlier-emitted) PE
                    # matmul, pinning its wait threshold at full-z instead
                    # of this half's last matmul; scheduled early, the h0
                    # tanh/sel chain overlaps the z h1 rounds.
                    h16 = work_pool.tile([128, 128], FP16, tag=f"h16_{h}",
                                         name=f"h16_{h}")
                    with tc.high_priority():
                        nc.scalar.activation(
                            h16[:, :], zps[h][:, :],
                            mybir.ActivationFunctionType.Tanh)
                    return h16

                def emit_sel(h16, h):
                    # PE: transpose back to state layout via selection matmul
                    hTp = psum_pool.tile([128, GH * 8], FP32,
                                         tag=f"hT{h}", name=f"hTp{h}")
                    nc.tensor.matmul(
                        hTp[:, :], h16[:, :], sel_sb[:, :],
                        start=True, stop=True, skip_group_check=True)
                    return hTp

                def emit_update(hTp, t, h):
                    s16_n = s16s[(t + 1) % 2][h]
                    nc.vector.tensor_add(s16_n[:, :], xhs[h][:, :], hTp[:, :])
                    s_r = s16_n[:, :].rearrange("p (g b) -> p g b", g=GH, b=8)
                    nc.vector.tensor_scalar_mul(
                        obuf_r[:, GH * h: GH * (h + 1), :, t],
                        s_r[:, :, :], ALPHA)

                def emit_xh(t, h):
                    # xh = 0.5 * s16 = x_t; consumed by s16' add after tanh
                    nc.vector.tensor_scalar_mul(
                        xhs[h][:, :], s16s[t % 2][h][:, :], ALPHA)

                zps_cur = emit_u(0)
                for t in range(tc_steps):
                    s16 = s16s[t % 2]
                    zps = zps_cur
                    emit_xh(t, 0)
                    emit_xh(t, 1)
                    emit_z(zps, s16, 0)
                    h16_0 = emit_tanh(zps, 0)     # ACT, runs under z h1
                    emit_z(zps, s16, 1)
                    h16_1 = emit_tanh(zps, 1)
                    # PE order: z h0 | z h1 | sel h0 | u | fillers | sel h1
                    hTp0 = emit_sel(h16_0, 0)
                    emit_update(hTp0, t, 0)
                    if t + 1 < tc_steps:
                        zps_cur = emit_u(t + 1)
                    filler(f"s{t}", FILLERS_PER_STEP)
                    hTp1 = emit_sel(h16_1, 1)
                    emit_update(hTp1, t, 1)

                for g in range(G):
                    nc.sync.dma_start(
                        x_dram_r[:, g, :, ds(ci * tc_steps, tc_steps)],
                        obuf_r[:, g, :, :],
                    )

            with tc.For_i(0, n_chunks, 1) as i:
                chunk_body(i)

    _split_excess_waits(nc)
    return nc


def kernel(Input, W_in, W):
    """Full inputs in, full output out. Shards batch over 8 NeuronCores."""
    global LAST_EXEC_NS, _CACHED_NC
    Input = np.ascontiguousarray(np.asarray(Input, dtype=np.float32))
    W_in = np.ascontiguousarray(np.asarray(W_in, dtype=np.float32))
    W = np.ascontiguousarray(np.asarray(W, dtype=np.float32))
    Bf, n_in, t_total = Input.shape
    n = W.shape[0]
    bc = Bf // N_CORES

    tc_steps = TC if t_total % TC == 0 else max(
        d for d in range(1, min(TC, t_total) + 1) if t_total % d == 0)
    if _CACHED_NC is None:
        _CACHED_NC = _build_nc(n=n, t_total=t_total, tc_steps=tc_steps,
                               n_in=n_in, bc=bc)
    nc = _CACHED_NC

    # leak factor folded into W: matmul operand is s = x + h = 2x, so W/2.
    # layout [p, (k, h, j, c)] = 0.5*W[128k+p, 512h+128j+c]
    w_r = np.ascontiguousarray(
        (ALPHA * W).reshape(8, 128, 2, 4, 128).transpose(1, 0, 2, 3, 4)
        .reshape(128, 8 * n)
    ).astype(np.float16)
    win16 = W_in.astype(np.float16)
    sel = np.zeros((128, 32), dtype=np.float16)
    for g_ in range(4):
        for b_ in range(8):
            sel[32 * g_ + b_, 8 * g_ + b_] = 1.0
    in_maps = []
    for c in range(N_CORES):
        inpT = np.ascontiguousarray(
            Input[c * bc:(c + 1) * bc].transpose(1, 2, 0)).astype(np.float16)
        in_maps.append({"w": w_r, "win": win16, "inpT": inpT, "sel": sel})

    trace = bool(int(os.environ.get("ESN_TRACE", "0")))
    res = run_bass_kernel_spmd(
        nc, in_maps, core_ids=list(range(N_CORES)), trace=trace)
    LAST_EXEC_NS = res.exec_time_ns

    out = np.concatenate([res.results[c]["xout"] for c in range(N_CORES)],
                         axis=0)
    return np.ascontiguousarray(out.astype(np.float32))
